# revision 22
# baseline (speedup 1.0000x reference)
"""Trainium2 Bass kernel for nn_ExactScalarArray.

Math: the reference computes, per (b, l):  prod_k reduce(c1*c2, p1+p2)
in an exact ring representation Z[w], w = e^{i pi/4}, then converts to
complex and sums over l with power-of-two alignment.  The ring embed
into C is a homomorphism and the reduce step is value-preserving, so
the whole thing equals

    out[b] = sum_l ( prod_k v1(b,l,k) * v2(b,l,k) ) * 2^{S(b,l)}
    v(c)   = (c0 + (c1+c3)/sqrt2) + i (c2 + (c1-c3)/sqrt2)
    S      = sum_k (p1+p2)

evaluated here in f32 complex arithmetic (max rel err vs the reference
~9e-6, measured).  Sharding: batch dim B=256 split across 8 cores; all
reduction axes (K, L) are core-local, so no collectives.

Host-side, the inputs (exact {0,1} values) are packed into dense bf16
component blocks inside one tensor per core: lossless, halves the HBM
traffic, and each chunk is a single DMA (one wait on the first
consumer; each ISA instruction has one sync-wait slot).
"""

import numpy as np

import concourse.bass as bass
import concourse.mybir as mybir
import concourse.tile as tile
from concourse.bass_utils import run_bass_kernel_spmd

# Problem shape (hardcoded per contract)
B, L, K = 256, 2048, 8
NCORES = 8
BC = B // NCORES            # 32 batch rows per core
NR = BC * L                 # 65536 (b,l) rows per core
P = 128                     # SBUF partitions
RPP = NR // P               # 512 rows per partition
TC = 128                    # rows-per-partition per chunk
NCHUNK = RPP // TC          # 4 chunks
NBLK = 10                   # bf16 blocks per chunk: c1_1,c1_3,c1_0,c1_2,
                            #   c2_1,c2_3,c2_0,c2_2, p1, p2  (each TC*K)
INV_SQRT2 = 0.7071067811865476

FP = mybir.dt.float32
BF = mybir.dt.bfloat16
I32 = mybir.dt.int32
ALU = mybir.AluOpType
AX = mybir.AxisListType

FK = TC * K              # one block, bf16 elements (1024)
FXF = TC * K * NBLK // 2  # packed chunk size in f32 units (5120)


def build_program(split_waits=True):
    nc = bass.Bass("TRN2", target_bir_lowering=False, debug=False,
                   num_devices=NCORES)
    xind = nc.dram_tensor("xin", [P, RPP * K * NBLK // 2], FP,
                          kind="ExternalInput").ap()
    outd = nc.dram_tensor("out", [P, 2], FP, kind="ExternalOutput").ap()
    with tile.TileContext(nc) as tc:
        build_kernel(nc, tc, xind, outd)
    if split_waits:
        _split_multiwait(nc)
    return nc


def _split_multiwait(nc):
    """Walrus allows one sync-wait per ISA instruction; hoist extras onto
    NOPs inserted just before the offender on the same engine."""
    k = 0
    for f in nc.m.functions:
        for bb in f.blocks:
            il = bb.instructions
            i = 0
            while i < len(il):
                inst = il[i]
                si = inst.sync_info
                if si is not None and si.on_wait and len(si.on_wait) > 1:
                    waits = list(si.on_wait)
                    for w in waits[:-1]:
                        nop = mybir.InstNoOp(name=f"WSPLIT-{k}", ins=[], outs=[])
                        k += 1
                        nop.engine = inst.engine
                        nop.sync_info = mybir.SyncInfo(on_wait=[w], on_update=[])
                        il.insert(i, nop)
                        i += 1
                    si.on_wait = waits[-1:]
                    inst.sync_info = si
                i += 1


def build_kernel(nc, tc, xind, outd):
    with (
        tc.tile_pool(name="io", bufs=2) as io_pool,
        tc.tile_pool(name="head", bufs=4) as head_pool,
        tc.tile_pool(name="work", bufs=1) as work_pool,
        tc.tile_pool(name="acc", bufs=1) as acc_pool,
    ):
        acc = acc_pool.tile([P, 2 * NCHUNK], FP)

        xts = []     # per-chunk xt tile objects
        progs = []   # per-chunk "all xt reads done" markers (DVE-written)
        for ch in range(NCHUNK):
            xt = io_pool.tile([P, FXF], FP, tag="xt")
            xts.append(xt)
            # Each ISA instruction has one sync-wait slot.  A reusing DMA
            # needs two waits (WAR vs the DVE readers of 2 chunks ago + WAW
            # vs that chunk's DMA); absorb each into its own tiny GPSIMD
            # fence copy issued ahead of the DMA on the same engine stream.
            fences = []
            if ch >= 2:
                fa = head_pool.tile([P, 1], FP, tag="fa")
                fb = head_pool.tile([P, 1], FP, tag="fb")
                fences.append(nc.gpsimd.tensor_copy(fa[:, :], progs[ch - 2][:, 0:1]))
                fences.append(nc.gpsimd.tensor_copy(fb[:, :], xts[ch - 2][:, 0:1]))
            dma = nc.gpsimd.dma_start(xt[:, :], xind[:, ch * FXF:(ch + 1) * FXF])
            for f in fences:
                tile.add_dep_helper(dma.ins, f.ins, False, "fence before dma")

            xb = xt[:, :].bitcast(BF)   # [P, NBLK*FK] bf16 blocks

            def blk(i, n=1):
                return xb[:, i * FK:(i + n) * FK]

            xt_readers = []

            # complexify both inputs.  t-tiles hold [t1|t2] = [c1+c3|c1-c3]
            # (exact small ints, bf16, 2x DVE mode); v-tiles hold [re|im] f32.
            # tj1 rotates through 4 buffers so the chunk's first consumer of
            # the DMA'd tile never also needs a same-engine WAR wait.
            tj1 = head_pool.tile([P, 2 * FK], BF, tag="tj1")
            tj2 = work_pool.tile([P, 2 * FK], BF, tag="tj2")
            v1 = work_pool.tile([P, 2 * FK], FP, tag="v1")
            v2 = work_pool.tile([P, 2 * FK], FP, tag="v2")
            xt_readers.append(nc.vector.tensor_tensor(
                tj1[:, 0:FK], blk(0), blk(1), ALU.add))
            xt_readers.append(nc.vector.tensor_tensor(
                tj1[:, FK:2 * FK], blk(0), blk(1), ALU.subtract))
            xt_readers.append(nc.vector.tensor_tensor(
                tj2[:, 0:FK], blk(4), blk(5), ALU.add))
            xt_readers.append(nc.vector.tensor_tensor(
                tj2[:, FK:2 * FK], blk(4), blk(5), ALU.subtract))
            xt_readers.append(nc.vector.scalar_tensor_tensor(
                v1[:, :], tj1[:, :], INV_SQRT2, blk(2, 2), ALU.mult, ALU.add))
            xt_readers.append(nc.vector.scalar_tensor_tensor(
                v2[:, :], tj2[:, :], INV_SQRT2, blk(6, 2), ALU.mult, ALU.add))

            # powers on GPSIMD (otherwise idle): S = sum_k (p1+p2), as a
            # strided add tree (GPSIMD tensor_reduce is partition-axis only)
            ps = work_pool.tile([P, FK], BF, tag="ps")
            xt_readers.append(nc.gpsimd.tensor_tensor(
                ps[:, :], blk(8), blk(9), ALU.add))
            pk = ps
            kwidth = FK
            while kwidth > TC:
                kwidth //= 2
                nk = work_pool.tile([P, kwidth], BF, tag=f"pk{kwidth}")
                nc.gpsimd.tensor_tensor(
                    nk[:, :], pk[:, 0:2 * kwidth:2], pk[:, 1:2 * kwidth:2],
                    ALU.add)
                pk = nk
            S_t = pk

            # pairwise product w = v1*v2, as [re|im] halves
            m12 = work_pool.tile([P, 2 * FK], FP, tag="m12")
            m34 = work_pool.tile([P, 2 * FK], FP, tag="m34")
            # v2 swapped halves: [im2|re2]
            v2sw = v2[:, :].rearrange("p (two n) -> p two n", two=2)[:, ::-1, :]
            nc.vector.tensor_tensor(m12[:, :], v1[:, :], v2[:, :], ALU.mult)
            nc.gpsimd.tensor_tensor(m34[:, :], v1[:, :], v2sw, ALU.mult)
            w = work_pool.tile([P, 2 * FK], FP, tag="w0")
            nc.vector.tensor_tensor(
                w[:, 0:FK], m12[:, 0:FK], m12[:, FK:2 * FK], ALU.subtract)
            nc.vector.tensor_tensor(
                w[:, FK:2 * FK], m34[:, 0:FK], m34[:, FK:2 * FK], ALU.add)

            # product tree over K: 8 -> 4 -> 2 -> 1.  w layout [P, 2, width]
            width = FK
            lvl = 0
            while width > TC:
                width //= 2
                lvl += 1
                wv = w[:, :].rearrange("p (two n) -> p two n", two=2)
                ev = wv[:, :, 0::2]
                ov = wv[:, :, 1::2]
                ovsw = ov[:, ::-1, :]
                q12 = work_pool.tile([P, 2 * width], FP, tag=f"q12_{lvl}")
                q34 = work_pool.tile([P, 2 * width], FP, tag=f"q34_{lvl}")
                nc.vector.tensor_tensor(
                    q12[:, :].rearrange("p (two n) -> p two n", two=2),
                    ev, ov, ALU.mult)
                qeng = nc.gpsimd if lvl == 1 else nc.vector
                qeng.tensor_tensor(
                    q34[:, :].rearrange("p (two n) -> p two n", two=2),
                    ev, ovsw, ALU.mult)
                nw = work_pool.tile([P, 2 * width], FP, tag=f"w{lvl}")
                nc.vector.tensor_tensor(
                    nw[:, 0:width], q12[:, 0:width], q12[:, width:2 * width],
                    ALU.subtract)
                nc.vector.tensor_tensor(
                    nw[:, width:2 * width], q34[:, 0:width],
                    q34[:, width:2 * width], ALU.add)
                w = nw

            prog = head_pool.tile([P, 1], FP, tag="prog")
            prog_copy = nc.vector.tensor_copy(prog[:, :], xt[:, 0:1])
            for r in xt_readers:
                tile.add_dep_helper(prog_copy.ins, r.ins, False,
                                    "prog after xt reads")
            progs.append(prog)

            # pw = 2^S exactly: (S+127)*2^23 is an exact f32 integer; convert
            # to i32 and reinterpret the bits as f32.
            pq = work_pool.tile([P, TC], FP, tag="pq")
            pwi = work_pool.tile([P, TC], I32, tag="pwi")
            nc.vector.tensor_scalar(
                pq[:, :], S_t[:, :], 127.0, float(1 << 23), ALU.add, ALU.mult)
            nc.vector.tensor_copy(pwi[:, :], pq[:, :])
            pw = pwi[:, :].bitcast(FP)
            pwb = pw.rearrange("p (one n) -> p one n", one=1).broadcast_to(
                (P, 2, TC))

            # sum_l w * 2^S -> this chunk's [re, im] accumulator columns
            dummy = work_pool.tile([P, 2 * TC], FP, tag="dummy")
            nc.vector.tensor_tensor(
                dummy[:, :].rearrange("p (two n) -> p two n", two=2),
                w[:, :].rearrange("p (two n) -> p two n", two=2), pwb, ALU.mult)
            nc.vector.tensor_reduce(
                acc[:, 2 * ch:2 * ch + 2],
                dummy[:, :].rearrange("p (two n) -> p two n", two=2),
                AX.X, ALU.add)

        outt = acc_pool.tile([P, 2], FP)
        nc.vector.tensor_reduce(
            outt[:, :], acc[:, :].rearrange("p (c two) -> p two c", two=2),
            AX.X, ALU.add)
        nc.gpsimd.dma_start(outd[:, :], outt[:, :])


_PROGRAM = None


def _get_program():
    global _PROGRAM
    if _PROGRAM is None:
        _PROGRAM = build_program()
    return _PROGRAM


def _to_bf16_bits(a):
    """f32 array of exact small ints -> uint16 bf16 bit patterns."""
    return (np.ascontiguousarray(a, dtype=np.float32).view(np.uint32) >> 16
            ).astype(np.uint16)


def pack_core_input(c1, c2, p1, p2):
    """Pack one core's inputs into [P, RPP*K*NBLK/2] f32 (bf16 bit blocks).

    Rows (b*L+l) map to partition r//RPP, chunk (r%RPP)//TC; within a chunk
    there are NBLK dense bf16 blocks of TC*K values each:
    c1_1, c1_3, c1_0, c1_2, c2_1, c2_3, c2_0, c2_2, p1, p2."""
    u = np.empty((P, NCHUNK, NBLK, TC * K), dtype=np.uint16)

    def comp(c, j):
        return _to_bf16_bits(c[..., j]).reshape(P, NCHUNK, TC * K)

    u[:, :, 0] = comp(c1, 1)
    u[:, :, 1] = comp(c1, 3)
    u[:, :, 2] = comp(c1, 0)
    u[:, :, 3] = comp(c1, 2)
    u[:, :, 4] = comp(c2, 1)
    u[:, :, 5] = comp(c2, 3)
    u[:, :, 6] = comp(c2, 0)
    u[:, :, 7] = comp(c2, 2)
    u[:, :, 8] = _to_bf16_bits(p1.astype(np.float32)).reshape(P, NCHUNK, TC * K)
    u[:, :, 9] = _to_bf16_bits(p2.astype(np.float32)).reshape(P, NCHUNK, TC * K)
    return u.reshape(P, -1).view(np.float32)


def kernel(coeffs1, coeffs2, power1, power2):
    nc = _get_program()
    in_maps = []
    for ci in range(NCORES):
        sl = slice(ci * BC, (ci + 1) * BC)
        in_maps.append({
            "xin": pack_core_input(coeffs1[sl], coeffs2[sl],
                                   power1[sl], power2[sl]),
        })
    res = run_bass_kernel_spmd(nc, in_maps, core_ids=list(range(NCORES)))
    outs = []
    for ci in range(NCORES):
        o = res.results[ci]["out"]  # [128, 2]
        outs.append(o.reshape(BC, P // BC, 2).sum(axis=1, dtype=np.float32))
    return np.concatenate(outs, axis=0).astype(np.float32)


# revision 23
# speedup vs baseline: 1.0969x; 1.0969x over previous
"""Trainium2 Bass kernel for nn_ExactScalarArray.

Math: the reference computes, per (b, l):  prod_k reduce(c1*c2, p1+p2)
in an exact ring representation Z[w], w = e^{i pi/4}, then converts to
complex and sums over l with power-of-two alignment.  The ring embed
into C is a homomorphism and the reduce step is value-preserving, so
the whole thing equals

    out[b] = sum_l ( prod_k v1(b,l,k) * v2(b,l,k) ) * 2^{S(b,l)}
    v(c)   = (c0 + (c1+c3)/sqrt2) + i (c2 + (c1-c3)/sqrt2)
    S      = sum_k (p1+p2)

evaluated here in f32 complex arithmetic (max rel err vs the reference
~9e-6, measured).  Sharding: batch dim B=256 split across 8 cores; all
reduction axes (K, L) are core-local, so no collectives.

Host-side, the inputs (exact {0,1} values) are packed into dense bf16
component blocks inside one tensor per core: lossless, halves the HBM
traffic, and each chunk is a single DMA (one wait on the first
consumer; each ISA instruction has one sync-wait slot).
"""

import numpy as np

import concourse.bass as bass
import concourse.mybir as mybir
import concourse.tile as tile
from concourse.bass_utils import run_bass_kernel_spmd

# Problem shape (hardcoded per contract)
B, L, K = 256, 2048, 8
NCORES = 8
BC = B // NCORES            # 32 batch rows per core
NR = BC * L                 # 65536 (b,l) rows per core
P = 128                     # SBUF partitions
RPP = NR // P               # 512 rows per partition
TC = 128                    # rows-per-partition per chunk
NCHUNK = RPP // TC          # 4 chunks
NBLK = 10                   # bf16 blocks per chunk: c1_1,c1_3,c1_0,c1_2,
                            #   c2_1,c2_3,c2_0,c2_2, p1, p2  (each TC*K)
INV_SQRT2 = 0.7071067811865476

FP = mybir.dt.float32
BF = mybir.dt.bfloat16
I32 = mybir.dt.int32
ALU = mybir.AluOpType
AX = mybir.AxisListType

FK = TC * K              # one block, bf16 elements (1024)
FXF = TC * K * NBLK // 2  # packed chunk size in f32 units (5120)


def build_program(split_waits=True):
    nc = bass.Bass("TRN2", target_bir_lowering=False, debug=False,
                   num_devices=NCORES)
    xind = nc.dram_tensor("xin", [P, RPP * K * NBLK // 2], FP,
                          kind="ExternalInput").ap()
    outd = nc.dram_tensor("out", [P, 2], FP, kind="ExternalOutput").ap()
    with tile.TileContext(nc) as tc:
        build_kernel(nc, tc, xind, outd)
    if split_waits:
        _split_multiwait(nc)
    return nc


def _split_multiwait(nc):
    """Walrus allows one sync-wait per ISA instruction; hoist extras onto
    NOPs inserted just before the offender on the same engine."""
    k = 0
    for f in nc.m.functions:
        for bb in f.blocks:
            il = bb.instructions
            i = 0
            while i < len(il):
                inst = il[i]
                si = inst.sync_info
                if si is not None and si.on_wait and len(si.on_wait) > 1:
                    waits = list(si.on_wait)
                    for w in waits[:-1]:
                        nop = mybir.InstNoOp(name=f"WSPLIT-{k}", ins=[], outs=[])
                        k += 1
                        nop.engine = inst.engine
                        nop.sync_info = mybir.SyncInfo(on_wait=[w], on_update=[])
                        il.insert(i, nop)
                        i += 1
                    si.on_wait = waits[-1:]
                    inst.sync_info = si
                i += 1


def build_kernel(nc, tc, xind, outd):
    with (
        tc.tile_pool(name="io", bufs=2) as io_pool,
        tc.tile_pool(name="head", bufs=4) as head_pool,
        tc.tile_pool(name="work", bufs=1) as work_pool,
        tc.tile_pool(name="acc", bufs=1) as acc_pool,
    ):
        acc = acc_pool.tile([P, 2 * NCHUNK], FP)

        xts = []     # per-chunk xt tile objects
        progs = []   # per-chunk "all xt reads done" markers (DVE-written)
        for ch in range(NCHUNK):
            xt = io_pool.tile([P, FXF], FP, tag="xt")
            xts.append(xt)
            # Each ISA instruction has one sync-wait slot.  A reusing DMA
            # needs two waits (WAR vs the DVE readers of 2 chunks ago + WAW
            # vs that chunk's DMA); absorb each into its own tiny GPSIMD
            # fence copy issued ahead of the DMA on the same engine stream.
            fences = []
            if ch >= 2:
                fa = head_pool.tile([P, 1], FP, tag="fa")
                fb = head_pool.tile([P, 1], FP, tag="fb")
                fences.append(nc.gpsimd.tensor_copy(fa[:, :], progs[ch - 2][:, 0:1]))
                fences.append(nc.gpsimd.tensor_copy(fb[:, :], xts[ch - 2][:, 0:1]))
            dma = nc.gpsimd.dma_start(xt[:, :], xind[:, ch * FXF:(ch + 1) * FXF])
            for f in fences:
                tile.add_dep_helper(dma.ins, f.ins, False, "fence before dma")

            xb = xt[:, :].bitcast(BF)   # [P, NBLK*FK] bf16 blocks

            def blk(i, n=1):
                return xb[:, i * FK:(i + n) * FK]

            xt_readers = []

            # complexify both inputs.  t-tiles hold [t1|t2] = [c1+c3|c1-c3]
            # (exact small ints, bf16, 2x DVE mode); v-tiles hold [re|im] f32.
            # tj1 rotates through 4 buffers so the chunk's first consumer of
            # the DMA'd tile never also needs a same-engine WAR wait.
            tj1 = head_pool.tile([P, 2 * FK], BF, tag="tj1")
            tj2 = work_pool.tile([P, 2 * FK], BF, tag="tj2")
            v1 = work_pool.tile([P, 2 * FK], FP, tag="v1")
            v2 = work_pool.tile([P, 2 * FK], FP, tag="v2")
            xt_readers.append(nc.vector.tensor_tensor(
                tj1[:, 0:FK], blk(0), blk(1), ALU.add))
            xt_readers.append(nc.vector.tensor_tensor(
                tj1[:, FK:2 * FK], blk(0), blk(1), ALU.subtract))
            xt_readers.append(nc.vector.tensor_tensor(
                tj2[:, 0:FK], blk(4), blk(5), ALU.add))
            xt_readers.append(nc.vector.tensor_tensor(
                tj2[:, FK:2 * FK], blk(4), blk(5), ALU.subtract))
            xt_readers.append(nc.vector.scalar_tensor_tensor(
                v1[:, :], tj1[:, :], INV_SQRT2, blk(2, 2), ALU.mult, ALU.add))
            xt_readers.append(nc.vector.scalar_tensor_tensor(
                v2[:, :], tj2[:, :], INV_SQRT2, blk(6, 2), ALU.mult, ALU.add))

            # powers on GPSIMD (otherwise idle): S = sum_k (p1+p2), as a
            # strided add tree (GPSIMD tensor_reduce is partition-axis only)
            ps = work_pool.tile([P, FK], BF, tag="ps")
            xt_readers.append(nc.gpsimd.tensor_tensor(
                ps[:, :], blk(8), blk(9), ALU.add))
            pk = ps
            kwidth = FK
            while kwidth > TC:
                kwidth //= 2
                nk = work_pool.tile([P, kwidth], BF, tag=f"pk{kwidth}")
                nc.gpsimd.tensor_tensor(
                    nk[:, :], pk[:, 0:2 * kwidth:2], pk[:, 1:2 * kwidth:2],
                    ALU.add)
                pk = nk
            S_t = pk

            # pairwise product w = v1*v2, as [re|im] halves
            m12 = work_pool.tile([P, 2 * FK], FP, tag="m12")
            m34 = work_pool.tile([P, 2 * FK], FP, tag="m34")
            # v2 swapped halves: [im2|re2]
            v2sw = v2[:, :].rearrange("p (two n) -> p two n", two=2)[:, ::-1, :]
            nc.vector.tensor_tensor(m12[:, :], v1[:, :], v2[:, :], ALU.mult)
            nc.vector.tensor_tensor(m34[:, :], v1[:, :], v2sw, ALU.mult)
            w = work_pool.tile([P, 2 * FK], FP, tag="w0")
            nc.vector.tensor_tensor(
                w[:, 0:FK], m12[:, 0:FK], m12[:, FK:2 * FK], ALU.subtract)
            nc.vector.tensor_tensor(
                w[:, FK:2 * FK], m34[:, 0:FK], m34[:, FK:2 * FK], ALU.add)

            # product tree over K: 8 -> 4 -> 2 -> 1.  w layout [P, 2, width]
            width = FK
            lvl = 0
            while width > TC:
                width //= 2
                lvl += 1
                wv = w[:, :].rearrange("p (two n) -> p two n", two=2)
                ev = wv[:, :, 0::2]
                ov = wv[:, :, 1::2]
                ovsw = ov[:, ::-1, :]
                q12 = work_pool.tile([P, 2 * width], FP, tag=f"q12_{lvl}")
                q34 = work_pool.tile([P, 2 * width], FP, tag=f"q34_{lvl}")
                nc.vector.tensor_tensor(
                    q12[:, :].rearrange("p (two n) -> p two n", two=2),
                    ev, ov, ALU.mult)
                nc.vector.tensor_tensor(
                    q34[:, :].rearrange("p (two n) -> p two n", two=2),
                    ev, ovsw, ALU.mult)
                nw = work_pool.tile([P, 2 * width], FP, tag=f"w{lvl}")
                nc.vector.tensor_tensor(
                    nw[:, 0:width], q12[:, 0:width], q12[:, width:2 * width],
                    ALU.subtract)
                nc.vector.tensor_tensor(
                    nw[:, width:2 * width], q34[:, 0:width],
                    q34[:, width:2 * width], ALU.add)
                w = nw

            prog = head_pool.tile([P, 1], FP, tag="prog")
            prog_copy = nc.vector.tensor_copy(prog[:, :], xt[:, 0:1])
            for r in xt_readers:
                tile.add_dep_helper(prog_copy.ins, r.ins, False,
                                    "prog after xt reads")
            progs.append(prog)

            # pw = 2^S exactly: (S+127)*2^23 is an exact f32 integer; convert
            # to i32 and reinterpret the bits as f32.
            pq = work_pool.tile([P, TC], FP, tag="pq")
            pwi = work_pool.tile([P, TC], I32, tag="pwi")
            nc.vector.tensor_scalar(
                pq[:, :], S_t[:, :], 127.0, float(1 << 23), ALU.add, ALU.mult)
            nc.vector.tensor_copy(pwi[:, :], pq[:, :])
            pw = pwi[:, :].bitcast(FP)
            pwb = pw.rearrange("p (one n) -> p one n", one=1).broadcast_to(
                (P, 2, TC))

            # sum_l w * 2^S -> this chunk's [re, im] accumulator columns
            dummy = work_pool.tile([P, 2 * TC], FP, tag="dummy")
            nc.vector.tensor_tensor(
                dummy[:, :].rearrange("p (two n) -> p two n", two=2),
                w[:, :].rearrange("p (two n) -> p two n", two=2), pwb, ALU.mult)
            nc.vector.tensor_reduce(
                acc[:, 2 * ch:2 * ch + 2],
                dummy[:, :].rearrange("p (two n) -> p two n", two=2),
                AX.X, ALU.add)

        outt = acc_pool.tile([P, 2], FP)
        nc.vector.tensor_reduce(
            outt[:, :], acc[:, :].rearrange("p (c two) -> p two c", two=2),
            AX.X, ALU.add)
        nc.gpsimd.dma_start(outd[:, :], outt[:, :])


_PROGRAM = None


def _get_program():
    global _PROGRAM
    if _PROGRAM is None:
        _PROGRAM = build_program()
    return _PROGRAM


def _to_bf16_bits(a):
    """f32 array of exact small ints -> uint16 bf16 bit patterns."""
    return (np.ascontiguousarray(a, dtype=np.float32).view(np.uint32) >> 16
            ).astype(np.uint16)


def pack_core_input(c1, c2, p1, p2):
    """Pack one core's inputs into [P, RPP*K*NBLK/2] f32 (bf16 bit blocks).

    Rows (b*L+l) map to partition r//RPP, chunk (r%RPP)//TC; within a chunk
    there are NBLK dense bf16 blocks of TC*K values each:
    c1_1, c1_3, c1_0, c1_2, c2_1, c2_3, c2_0, c2_2, p1, p2."""
    u = np.empty((P, NCHUNK, NBLK, TC * K), dtype=np.uint16)

    def comp(c, j):
        return _to_bf16_bits(c[..., j]).reshape(P, NCHUNK, TC * K)

    u[:, :, 0] = comp(c1, 1)
    u[:, :, 1] = comp(c1, 3)
    u[:, :, 2] = comp(c1, 0)
    u[:, :, 3] = comp(c1, 2)
    u[:, :, 4] = comp(c2, 1)
    u[:, :, 5] = comp(c2, 3)
    u[:, :, 6] = comp(c2, 0)
    u[:, :, 7] = comp(c2, 2)
    u[:, :, 8] = _to_bf16_bits(p1.astype(np.float32)).reshape(P, NCHUNK, TC * K)
    u[:, :, 9] = _to_bf16_bits(p2.astype(np.float32)).reshape(P, NCHUNK, TC * K)
    return u.reshape(P, -1).view(np.float32)


def kernel(coeffs1, coeffs2, power1, power2):
    nc = _get_program()
    in_maps = []
    for ci in range(NCORES):
        sl = slice(ci * BC, (ci + 1) * BC)
        in_maps.append({
            "xin": pack_core_input(coeffs1[sl], coeffs2[sl],
                                   power1[sl], power2[sl]),
        })
    res = run_bass_kernel_spmd(nc, in_maps, core_ids=list(range(NCORES)))
    outs = []
    for ci in range(NCORES):
        o = res.results[ci]["out"]  # [128, 2]
        outs.append(o.reshape(BC, P // BC, 2).sum(axis=1, dtype=np.float32))
    return np.concatenate(outs, axis=0).astype(np.float32)


# revision 24
# speedup vs baseline: 1.1936x; 1.0881x over previous
"""Trainium2 Bass kernel for nn_ExactScalarArray.

Math: the reference computes, per (b, l):  prod_k reduce(c1*c2, p1+p2)
in an exact ring representation Z[w], w = e^{i pi/4}, then converts to
complex and sums over l with power-of-two alignment.  The ring embed
into C is a homomorphism and the reduce step is value-preserving, so
the whole thing equals

    out[b] = sum_l ( prod_k v1(b,l,k) * v2(b,l,k) ) * 2^{S(b,l)}
    v(c)   = (c0 + (c1+c3)/sqrt2) + i (c2 + (c1-c3)/sqrt2)
    S      = sum_k (p1+p2)

evaluated here in f32 complex arithmetic (max rel err vs the reference
~9e-6, measured).  Sharding: batch dim B=256 split across 8 cores; all
reduction axes (K, L) are core-local, so no collectives.

Host-side, the inputs (exact {0,1} values) are packed into dense bf16
component blocks inside one tensor per core: lossless, halves the HBM
traffic, and each chunk is a single DMA (one wait on the first
consumer; each ISA instruction has one sync-wait slot).
"""

import numpy as np

import concourse.bass as bass
import concourse.mybir as mybir
import concourse.tile as tile
from concourse.bass_utils import run_bass_kernel_spmd

# Problem shape (hardcoded per contract)
B, L, K = 256, 2048, 8
NCORES = 8
BC = B // NCORES            # 32 batch rows per core
NR = BC * L                 # 65536 (b,l) rows per core
P = 128                     # SBUF partitions
RPP = NR // P               # 512 rows per partition
TC = 128                    # rows-per-partition per chunk
NCHUNK = RPP // TC          # 4 chunks
NBLK = 10                   # bf16 blocks per chunk: c1_1,c1_3,c1_0,c1_2,
                            #   c2_1,c2_3,c2_0,c2_2, p1, p2  (each TC*K)
INV_SQRT2 = 0.7071067811865476

FP = mybir.dt.float32
BF = mybir.dt.bfloat16
I32 = mybir.dt.int32
ALU = mybir.AluOpType
AX = mybir.AxisListType

FK = TC * K              # one block, bf16 elements (1024)
FXF = TC * K * NBLK // 2  # packed chunk size in f32 units (5120)


def build_program(split_waits=True):
    nc = bass.Bass("TRN2", target_bir_lowering=False, debug=False,
                   num_devices=NCORES)
    xind = nc.dram_tensor("xin", [P, RPP * K * NBLK // 2], FP,
                          kind="ExternalInput").ap()
    outd = nc.dram_tensor("out", [P, 2], FP, kind="ExternalOutput").ap()
    with tile.TileContext(nc) as tc:
        build_kernel(nc, tc, xind, outd)
    if split_waits:
        _split_multiwait(nc)
    return nc


def _split_multiwait(nc):
    """Walrus allows one sync-wait per ISA instruction; hoist extras onto
    NOPs inserted just before the offender on the same engine."""
    k = 0
    for f in nc.m.functions:
        for bb in f.blocks:
            il = bb.instructions
            i = 0
            while i < len(il):
                inst = il[i]
                si = inst.sync_info
                if si is not None and si.on_wait and len(si.on_wait) > 1:
                    waits = list(si.on_wait)
                    for w in waits[:-1]:
                        nop = mybir.InstNoOp(name=f"WSPLIT-{k}", ins=[], outs=[])
                        k += 1
                        nop.engine = inst.engine
                        nop.sync_info = mybir.SyncInfo(on_wait=[w], on_update=[])
                        il.insert(i, nop)
                        i += 1
                    si.on_wait = waits[-1:]
                    inst.sync_info = si
                i += 1


def build_kernel(nc, tc, xind, outd):
    with (
        tc.tile_pool(name="io", bufs=2) as io_pool,
        tc.tile_pool(name="head", bufs=4) as head_pool,
        tc.tile_pool(name="work", bufs=1) as work_pool,
        tc.tile_pool(name="acc", bufs=1) as acc_pool,
    ):
        acc = acc_pool.tile([P, 2 * NCHUNK], FP)

        xts = []     # per-chunk xt tile objects
        progs = []   # per-chunk "all xt reads done" markers (DVE-written)
        for ch in range(NCHUNK):
            xt = io_pool.tile([P, FXF], FP, tag="xt")
            xts.append(xt)
            # Each ISA instruction has one sync-wait slot.  A reusing DMA
            # needs two waits (WAR vs the DVE readers of 2 chunks ago + WAW
            # vs that chunk's DMA); absorb each into its own tiny GPSIMD
            # fence copy issued ahead of the DMA on the same engine stream.
            fences = []
            if ch >= 2:
                fa = head_pool.tile([P, 1], FP, tag="fa")
                fb = head_pool.tile([P, 1], FP, tag="fb")
                fences.append(nc.gpsimd.tensor_copy(fa[:, :], progs[ch - 2][:, 0:1]))
                fences.append(nc.gpsimd.tensor_copy(fb[:, :], xts[ch - 2][:, 0:1]))
            dma = nc.gpsimd.dma_start(xt[:, :], xind[:, ch * FXF:(ch + 1) * FXF])
            for f in fences:
                tile.add_dep_helper(dma.ins, f.ins, False, "fence before dma")

            xb = xt[:, :].bitcast(BF)   # [P, NBLK*FK] bf16 blocks

            def blk(i, n=1):
                return xb[:, i * FK:(i + n) * FK]

            xt_readers = []

            # complexify both inputs.  t-tiles hold [t1|t2] = [c1+c3|c1-c3]
            # (exact small ints, bf16, 2x DVE mode); v-tiles hold [re|im] f32.
            # tj1 rotates through 4 buffers so the chunk's first consumer of
            # the DMA'd tile never also needs a same-engine WAR wait.
            tj1 = head_pool.tile([P, 2 * FK], BF, tag="tj1")
            tj2 = work_pool.tile([P, 2 * FK], BF, tag="tj2")
            v1 = work_pool.tile([P, 2 * FK], FP, tag="v1")
            v2 = work_pool.tile([P, 2 * FK], FP, tag="v2")
            xt_readers.append(nc.vector.tensor_tensor(
                tj1[:, 0:FK], blk(0), blk(1), ALU.add))
            xt_readers.append(nc.vector.tensor_tensor(
                tj1[:, FK:2 * FK], blk(0), blk(1), ALU.subtract))
            xt_readers.append(nc.vector.tensor_tensor(
                tj2[:, 0:FK], blk(4), blk(5), ALU.add))
            xt_readers.append(nc.vector.tensor_tensor(
                tj2[:, FK:2 * FK], blk(4), blk(5), ALU.subtract))
            xt_readers.append(nc.vector.scalar_tensor_tensor(
                v1[:, :], tj1[:, :], INV_SQRT2, blk(2, 2), ALU.mult, ALU.add))
            xt_readers.append(nc.vector.scalar_tensor_tensor(
                v2[:, :], tj2[:, :], INV_SQRT2, blk(6, 2), ALU.mult, ALU.add))

            # powers on GPSIMD (otherwise idle): S = sum_k (p1+p2), as a
            # strided add tree (GPSIMD tensor_reduce is partition-axis only)
            ps = work_pool.tile([P, FK], BF, tag="ps")
            xt_readers.append(nc.vector.tensor_tensor(
                ps[:, :], blk(8), blk(9), ALU.add))
            pk = ps
            kwidth = FK
            while kwidth > TC:
                kwidth //= 2
                nk = work_pool.tile([P, kwidth], BF, tag=f"pk{kwidth}")
                nc.vector.tensor_tensor(
                    nk[:, :], pk[:, 0:2 * kwidth:2], pk[:, 1:2 * kwidth:2],
                    ALU.add)
                pk = nk
            S_t = pk

            # pairwise product w = v1*v2, as [re|im] halves
            m12 = work_pool.tile([P, 2 * FK], FP, tag="m12")
            m34 = work_pool.tile([P, 2 * FK], FP, tag="m34")
            # v2 swapped halves: [im2|re2]
            v2sw = v2[:, :].rearrange("p (two n) -> p two n", two=2)[:, ::-1, :]
            nc.vector.tensor_tensor(m12[:, :], v1[:, :], v2[:, :], ALU.mult)
            nc.vector.tensor_tensor(m34[:, :], v1[:, :], v2sw, ALU.mult)
            w = work_pool.tile([P, 2 * FK], FP, tag="w0")
            nc.vector.tensor_tensor(
                w[:, 0:FK], m12[:, 0:FK], m12[:, FK:2 * FK], ALU.subtract)
            nc.vector.tensor_tensor(
                w[:, FK:2 * FK], m34[:, 0:FK], m34[:, FK:2 * FK], ALU.add)

            # product tree over K: 8 -> 4 -> 2 -> 1.  w layout [P, 2, width]
            width = FK
            lvl = 0
            while width > TC:
                width //= 2
                lvl += 1
                wv = w[:, :].rearrange("p (two n) -> p two n", two=2)
                ev = wv[:, :, 0::2]
                ov = wv[:, :, 1::2]
                ovsw = ov[:, ::-1, :]
                q12 = work_pool.tile([P, 2 * width], FP, tag=f"q12_{lvl}")
                q34 = work_pool.tile([P, 2 * width], FP, tag=f"q34_{lvl}")
                nc.vector.tensor_tensor(
                    q12[:, :].rearrange("p (two n) -> p two n", two=2),
                    ev, ov, ALU.mult)
                nc.vector.tensor_tensor(
                    q34[:, :].rearrange("p (two n) -> p two n", two=2),
                    ev, ovsw, ALU.mult)
                nw = work_pool.tile([P, 2 * width], FP, tag=f"w{lvl}")
                nc.vector.tensor_tensor(
                    nw[:, 0:width], q12[:, 0:width], q12[:, width:2 * width],
                    ALU.subtract)
                nc.vector.tensor_tensor(
                    nw[:, width:2 * width], q34[:, 0:width],
                    q34[:, width:2 * width], ALU.add)
                w = nw

            prog = head_pool.tile([P, 1], FP, tag="prog")
            prog_copy = nc.vector.tensor_copy(prog[:, :], xt[:, 0:1])
            for r in xt_readers:
                tile.add_dep_helper(prog_copy.ins, r.ins, False,
                                    "prog after xt reads")
            progs.append(prog)

            # pw = 2^S exactly: (S+127)*2^23 is an exact f32 integer; convert
            # to i32 and reinterpret the bits as f32.
            pq = work_pool.tile([P, TC], FP, tag="pq")
            pwi = work_pool.tile([P, TC], I32, tag="pwi")
            nc.vector.tensor_scalar(
                pq[:, :], S_t[:, :], 127.0, float(1 << 23), ALU.add, ALU.mult)
            nc.vector.tensor_copy(pwi[:, :], pq[:, :])
            pw = pwi[:, :].bitcast(FP)
            pwb = pw.rearrange("p (one n) -> p one n", one=1).broadcast_to(
                (P, 2, TC))

            # sum_l w * 2^S -> this chunk's [re, im] accumulator columns
            dummy = work_pool.tile([P, 2 * TC], FP, tag="dummy")
            nc.vector.tensor_tensor(
                dummy[:, :].rearrange("p (two n) -> p two n", two=2),
                w[:, :].rearrange("p (two n) -> p two n", two=2), pwb, ALU.mult)
            nc.vector.tensor_reduce(
                acc[:, 2 * ch:2 * ch + 2],
                dummy[:, :].rearrange("p (two n) -> p two n", two=2),
                AX.X, ALU.add)

        outt = acc_pool.tile([P, 2], FP)
        nc.vector.tensor_reduce(
            outt[:, :], acc[:, :].rearrange("p (c two) -> p two c", two=2),
            AX.X, ALU.add)
        nc.gpsimd.dma_start(outd[:, :], outt[:, :])


_PROGRAM = None


def _get_program():
    global _PROGRAM
    if _PROGRAM is None:
        _PROGRAM = build_program()
    return _PROGRAM


def _to_bf16_bits(a):
    """f32 array of exact small ints -> uint16 bf16 bit patterns."""
    return (np.ascontiguousarray(a, dtype=np.float32).view(np.uint32) >> 16
            ).astype(np.uint16)


def pack_core_input(c1, c2, p1, p2):
    """Pack one core's inputs into [P, RPP*K*NBLK/2] f32 (bf16 bit blocks).

    Rows (b*L+l) map to partition r//RPP, chunk (r%RPP)//TC; within a chunk
    there are NBLK dense bf16 blocks of TC*K values each:
    c1_1, c1_3, c1_0, c1_2, c2_1, c2_3, c2_0, c2_2, p1, p2."""
    u = np.empty((P, NCHUNK, NBLK, TC * K), dtype=np.uint16)

    def comp(c, j):
        return _to_bf16_bits(c[..., j]).reshape(P, NCHUNK, TC * K)

    u[:, :, 0] = comp(c1, 1)
    u[:, :, 1] = comp(c1, 3)
    u[:, :, 2] = comp(c1, 0)
    u[:, :, 3] = comp(c1, 2)
    u[:, :, 4] = comp(c2, 1)
    u[:, :, 5] = comp(c2, 3)
    u[:, :, 6] = comp(c2, 0)
    u[:, :, 7] = comp(c2, 2)
    u[:, :, 8] = _to_bf16_bits(p1.astype(np.float32)).reshape(P, NCHUNK, TC * K)
    u[:, :, 9] = _to_bf16_bits(p2.astype(np.float32)).reshape(P, NCHUNK, TC * K)
    return u.reshape(P, -1).view(np.float32)


def kernel(coeffs1, coeffs2, power1, power2):
    nc = _get_program()
    in_maps = []
    for ci in range(NCORES):
        sl = slice(ci * BC, (ci + 1) * BC)
        in_maps.append({
            "xin": pack_core_input(coeffs1[sl], coeffs2[sl],
                                   power1[sl], power2[sl]),
        })
    res = run_bass_kernel_spmd(nc, in_maps, core_ids=list(range(NCORES)))
    outs = []
    for ci in range(NCORES):
        o = res.results[ci]["out"]  # [128, 2]
        outs.append(o.reshape(BC, P // BC, 2).sum(axis=1, dtype=np.float32))
    return np.concatenate(outs, axis=0).astype(np.float32)
